# revision 1
# baseline (speedup 1.0000x reference)
"""Distributed GNN message-passing kernel for one TRN2 chip (8 NeuronCores).

Math (matches the reference):
    h = spmm(relu(x@W1+b1)); h = spmm(relu(h@W2+b2)); h = spmm(relu(h@W3+b3))
    g = mean(h, 0); o = relu(g@Wf1+bf1); r = sigmoid(o@Wf2+bf2)
with spmm(h)[i] = sum_{e: dst[e]=i} w[e] * h[src[e]].

Sharding: nodes block-partitioned over the 8 cores (core r owns dst rows
[r*6250, (r+1)*6250)); each edge is assigned to the core owning its dst.
Layer-1 dense is computed replicated from the full x (no comm needed);
layers 2-3 dense are computed locally on owned rows and exchanged with an
fp16 AllGather ("halo exchange" of all Z rows - the graph is uniform random
so every core needs essentially every row). The spmm itself is a per-edge
SWDGE dma_gather of Z rows (fp16, 256B descriptors, spread over the 16 DMA
engines) feeding per-128-edge one-hot segment matmuls on the TensorEngine
that accumulate H^T windows in PSUM (scatter side is free: PSUM
accumulation). Readout: local column-sum + AllReduce + tiny MLP head.

Host-side preprocessing builds the data-dependent static schedule: edges
sorted by (core, src-half, dst-window); each (half, window) group padded to
a multiple of 128 with dummy edges (idx 0, dstoff -1, w 0); subtile counts
shared across cores (max over cores) so all 8 cores run one SPMD graph.
src indices are split in two halves (< / >= 25000) because gather indices
are int16.
"""
import sys
sys.path.insert(0, '/opt/trn_rl_repo')
from dataclasses import dataclass
import numpy as np

import concourse.bass as bass
import concourse.bacc as bacc
import concourse.tile as tile
from concourse import mybir, library_config
from concourse.bass_utils import run_bass_kernel_spmd

P = 128
F = 128
NC = 8
FP16 = mybir.dt.float16
F32 = mybir.dt.float32
I16 = mybir.dt.int16


@dataclass
class Cfg:
    N: int = 50000
    CALL: int = 896  # idxs/dma_gather call: 57 descs x 2 bufs <= 128 DGE FIFO
    N_LAYERS: int = 3

    @property
    def NPC(self):
        return self.N // NC

    @property
    def NW(self):
        return (self.NPC + P - 1) // P

    @property
    def HALF(self):
        return self.N // 2

    @property
    def ROWW(self):
        return (self.N + P - 1) // P


def preprocess(cfg, edge_src, edge_dst, edge_weight):
    """Per-core gather indices / one-hot metadata + shared subtile schedule."""
    NPC, NW, HALF = cfg.NPC, cfg.NW, cfg.HALF
    edge_src = np.asarray(edge_src).astype(np.int64)
    edge_dst = np.asarray(edge_dst).astype(np.int64)
    edge_weight = np.asarray(edge_weight).astype(np.float32)
    core = edge_dst // NPC
    local = edge_dst % NPC
    win = local // P
    dstoff = local % P
    half = (edge_src >= HALF).astype(np.int64)
    srcoff = np.where(half == 1, edge_src - HALF, edge_src)

    order = np.lexsort((win, half, core))
    core_s, half_s, win_s = core[order], half[order], win[order]
    srcoff_s, dstoff_s, w_s = srcoff[order], dstoff[order], edge_weight[order]

    counts = np.zeros((NC, 2, NW), dtype=np.int64)
    np.add.at(counts, (core_s, half_s, win_s), 1)
    nsub = np.ceil(counts / P).astype(np.int64).max(axis=0)  # [2, NW]
    nsub = np.maximum(nsub, 1)

    sub_base = np.zeros((2, NW), dtype=np.int64)
    acc = 0
    for h in range(2):
        for w in range(NW):
            sub_base[h, w] = acc
            acc += nsub[h, w]
    S = acc
    nA = int(nsub[0].sum()) * P
    nB = int(nsub[1].sum()) * P

    grp_start = np.zeros(NC * 2 * NW, dtype=np.int64)
    np.cumsum(counts.ravel()[:-1], out=grp_start[1:])
    grp_start = grp_start.reshape(NC, 2, NW)

    per_core = []
    for c in range(NC):
        idx_all = np.zeros(S * P, dtype=np.int64)
        off_all = np.full(S * P, -1.0, dtype=np.float32)
        w_all = np.zeros(S * P, dtype=np.float32)
        for h in range(2):
            for w in range(NW):
                cnt = counts[c, h, w]
                g0 = grp_start[c, h, w]
                s0 = sub_base[h, w] * P
                idx_all[s0:s0 + cnt] = srcoff_s[g0:g0 + cnt]
                off_all[s0:s0 + cnt] = dstoff_s[g0:g0 + cnt]
                w_all[s0:s0 + cnt] = w_s[g0:g0 + cnt]
        idxA = idx_all[:nA].astype(np.int16)
        idxB = idx_all[nA:].astype(np.int16)
        # dma_gather idx layout: idx j -> partition j%16, col j//16,
        # replicated across the 8 gpsimd core groups
        wrapA = np.tile(idxA.reshape(-1, 16).T, (8, 1)).astype(np.int16)
        wrapB = np.tile(idxB.reshape(-1, 16).T, (8, 1)).astype(np.int16)
        off_cols = np.ascontiguousarray(off_all.reshape(S, P).T)
        w_cols = np.ascontiguousarray(w_all.reshape(S, P).T)
        per_core.append(dict(idxA=wrapA, idxB=wrapB,
                             dstoff=off_cols, wcol=w_cols))
    sched = dict(nsub=nsub, sub_base=sub_base, S=S, nA=nA, nB=nB)
    return sched, per_core


def build(cfg, sched):
    """Build the (SPMD, shared by all 8 cores) Bacc graph."""
    N, NPC, NW, HALF, ROWW, CALL = (cfg.N, cfg.NPC, cfg.NW, cfg.HALF,
                                    cfg.ROWW, cfg.CALL)
    nsub, S, nA, nB = sched["nsub"], sched["S"], sched["nA"], sched["nB"]
    nc = bacc.Bacc('TRN2', target_bir_lowering=False, debug=False,
                   num_devices=NC)

    x_t = nc.dram_tensor("x_t", [P, N], FP16, kind="ExternalInput")
    idxA_d = nc.dram_tensor("idxA", [P, nA // 16], I16, kind="ExternalInput")
    idxB_d = nc.dram_tensor("idxB", [P, nB // 16], I16, kind="ExternalInput")
    dstoff_d = nc.dram_tensor("dstoff", [P, S], F32, kind="ExternalInput")
    wcol_d = nc.dram_tensor("wcol", [P, S], F32, kind="ExternalInput")
    W_d = [nc.dram_tensor(f"W{l}", [F, F], FP16, kind="ExternalInput")
           for l in (1, 2, 3)]
    b_d = [nc.dram_tensor(f"b{l}", [1, F], FP16, kind="ExternalInput")
           for l in (1, 2, 3)]
    Wf1_d = nc.dram_tensor("Wf1", [F, 32], FP16, kind="ExternalInput")
    bf1_d = nc.dram_tensor("bf1", [32, 1], FP16, kind="ExternalInput")
    Wf2_d = nc.dram_tensor("Wf2", [32, 1], FP16, kind="ExternalInput")
    bf2_d = nc.dram_tensor("bf2", [1, 1], FP16, kind="ExternalInput")
    iota_d = nc.dram_tensor("iotat", [P, P], FP16, kind="ExternalInput")
    out_d = nc.dram_tensor("out", [1, 1], F32, kind="ExternalOutput")

    with tile.TileContext(nc) as tc:
        with tc.tile_pool(name="resident", bufs=1) as res, \
             tc.tile_pool(name="xstream", bufs=4) as xs, \
             tc.tile_pool(name="stage", bufs=2) as stg, \
             tc.tile_pool(name="onehot", bufs=4) as ohp, \
             tc.tile_pool(name="zrow", bufs=4) as zrp, \
             tc.tile_pool(name="psum", bufs=2, space="PSUM") as psp, \
             tc.tile_pool(name="dram", bufs=1, space="DRAM") as drm:

            nc.gpsimd.load_library(library_config.mlp)

            idxA = res.tile([P, nA // 16], I16)
            idxB = res.tile([P, nB // 16], I16)
            dstoff = res.tile([P, S], F32)
            wcol = res.tile([P, S], F32)
            nc.sync.dma_start(idxA[:], idxA_d[:])
            nc.sync.dma_start(idxB[:], idxB_d[:])
            nc.sync.dma_start(dstoff[:], dstoff_d[:])
            nc.sync.dma_start(wcol[:], wcol_d[:])
            Ws = []
            for l in range(3):
                t = res.tile([F, F], FP16, tag=f"W{l}", name=f"Wsb{l}")
                nc.sync.dma_start(t[:], W_d[l][:])
                Ws.append(t)
            bs = []
            for l in range(3):
                t = res.tile([1, F], FP16, tag=f"b{l}", name=f"bsb{l}")
                nc.sync.dma_start(t[:], b_d[l][:])
                bs.append(t)
            Wf1 = res.tile([F, 32], FP16)
            nc.sync.dma_start(Wf1[:], Wf1_d[:])
            bf1 = res.tile([32, 1], FP16)
            nc.sync.dma_start(bf1[:], bf1_d[:])
            Wf2 = res.tile([32, 1], FP16)
            nc.sync.dma_start(Wf2[:], Wf2_d[:])
            bf2 = res.tile([1, 1], FP16)
            nc.sync.dma_start(bf2[:], bf2_d[:])
            iota = res.tile([P, P], FP16)
            nc.sync.dma_start(iota[:], iota_d[:])
            ones_row = res.tile([1, P], FP16)
            nc.vector.memset(ones_row[:], 1.0)

            # H^T accumulator for the current layer [feat, local nodes]
            HT = res.tile([P, NPC], FP16)

            # AllGather/AllReduce outputs in Shared scratchpad (peers write
            # directly); Tile tracks raw dram tensors by name.
            Z_full = [drm.tile([N, F], FP16, tag="Zfull0", name="Zfull0")]
            for l in (1, 2):
                Z_full.append(nc.dram_tensor(f"Zfull{l}", [N, F], FP16,
                                             kind="Internal",
                                             addr_space="Shared").ap())
            Z_shard = [drm.tile([NPC, F], FP16, tag=f"Zshard{l}",
                                name=f"Zshard{l}") for l in range(2)]
            g_in = drm.tile([P, 1], F32)
            g_out = nc.dram_tensor("g_out", [P, 1], F32, kind="Internal",
                                   addr_space="Shared").ap()

            # ---- layer-1 dense (replicated over all N rows) ------------
            GB = 4  # row-windows per PSUM bank / DMA batch
            for rg in range(0, ROWW, GB):
                r0 = rg * P
                gw = min(GB, ROWW - rg)
                rows_tot = min(GB * P, N - r0)
                xt_tile = xs.tile([P, GB * P], FP16, tag="xt")
                nc.sync.dma_start(xt_tile[:, :rows_tot], x_t[:, r0:r0 + rows_tot])
                ps = psp.tile([P, GB, F], F32, tag="dense", bufs=2)
                for w in range(gw):
                    rows = min(P, rows_tot - w * P)
                    nc.tensor.matmul(ps[:rows, w, :],
                                     xt_tile[:, w * P:w * P + rows], Ws[0][:],
                                     start=True, stop=False)
                    nc.tensor.matmul(ps[:rows, w, :], ones_row[:, :rows],
                                     bs[0][:], start=False, stop=True)
                zrow = zrp.tile([P, GB, F], FP16, tag="zrow")
                if rows_tot == GB * P:
                    nc.scalar.activation(
                        zrow[:].rearrange("p w f -> p (w f)"),
                        ps[:].rearrange("p w f -> p (w f)"),
                        mybir.ActivationFunctionType.Relu)
                    dst = Z_full[0][r0:r0 + GB * P, :].rearrange(
                        "(w p) f -> p w f", p=P)
                    nc.sync.dma_start(dst, zrow[:])
                else:
                    for w in range(gw):
                        rows = min(P, rows_tot - w * P)
                        nc.scalar.activation(
                            zrow[:rows, w, :], ps[:rows, w, :],
                            mybir.ActivationFunctionType.Relu)
                        nc.sync.dma_start(
                            Z_full[0][r0 + w * P:r0 + w * P + rows, :],
                            zrow[:rows, w, :])

            # ---- spmm layers -------------------------------------------
            for l in range(cfg.N_LAYERS):
                zf = Z_full[l]
                for h in range(2):
                    idx_sb = idxA if h == 0 else idxB
                    n_idx = nA if h == 0 else nB
                    src_ap = zf[h * HALF:(h + 1) * HALF, :]
                    n_sub_pass = n_idx // P
                    stages = []  # (tile, first_subtile, n_sub)
                    done = 0
                    while done < n_sub_pass:
                        k = min(CALL // P, n_sub_pass - done)
                        st = stg.tile([P, CALL // P, F], FP16, tag="gst")
                        nc.gpsimd.dma_gather(
                            out_ap=st[:, :k, :], in_ap=src_ap,
                            idxs_ap=idx_sb[:, done * P // 16:(done + k) * P // 16],
                            num_idxs=k * P, num_idxs_reg=k * P, elem_size=F)
                        stages.append((st, done, k))
                        done += k
                    si = 0
                    sg = 0
                    WG = 4  # windows per PSUM bank (4 x 512B = one bank)
                    for wg in range(0, NW, WG):
                        gw = min(WG, NW - wg)
                        ps = psp.tile([P, WG, P], F32, tag="spmm", bufs=3)
                        for wi in range(gw):
                            w = wg + wi
                            ns = int(nsub[h, w])
                            for k in range(ns):
                                s_glob = int(sched["sub_base"][h, w]) + k
                                st, s0, sk = stages[sg]
                                loc = si - s0
                                oh = ohp.tile([P, P], FP16, tag="oh")
                                # one-hot row e = w[e] * (iota == dstoff[e])
                                nc.vector.tensor_scalar(
                                    oh[:], iota[:],
                                    dstoff[:, s_glob:s_glob + 1],
                                    wcol[:, s_glob:s_glob + 1],
                                    mybir.AluOpType.is_equal,
                                    mybir.AluOpType.mult)
                                # H^T[:, window] += G^T(e,f) @ OH(e,seg)
                                nc.tensor.matmul(ps[:, wi, :], st[:, loc, :],
                                                 oh[:], start=(k == 0),
                                                 stop=(k == ns - 1))
                                si += 1
                                if si - s0 >= sk:
                                    sg += 1
                        c0 = wg * P
                        cols = min(WG * P, NPC - c0)
                        src = ps[:].rearrange("p w f -> p (w f)")[:, :cols]
                        if h == 0:
                            nc.vector.tensor_copy(HT[:, c0:c0 + cols], src)
                        else:
                            nc.vector.tensor_tensor(
                                HT[:, c0:c0 + cols], HT[:, c0:c0 + cols],
                                src, mybir.AluOpType.add)

                if l < cfg.N_LAYERS - 1:
                    # local dense l+2 on owned rows, then AllGather of Z
                    for rg in range(0, NW, GB):
                        r0 = rg * P
                        gw = min(GB, NW - rg)
                        rows_tot = min(GB * P, NPC - r0)
                        ps = psp.tile([P, GB, F], F32, tag="dense", bufs=2,
                                      name="ps_d2")
                        for w in range(gw):
                            rows = min(P, rows_tot - w * P)
                            nc.tensor.matmul(
                                ps[:rows, w, :],
                                HT[:, r0 + w * P:r0 + w * P + rows],
                                Ws[l + 1][:], start=True, stop=False)
                            nc.tensor.matmul(ps[:rows, w, :],
                                             ones_row[:, :rows], bs[l + 1][:],
                                             start=False, stop=True)
                        zrow = zrp.tile([P, GB, F], FP16, tag="zrow2")
                        if rows_tot == GB * P:
                            nc.scalar.activation(
                                zrow[:].rearrange("p w f -> p (w f)"),
                                ps[:].rearrange("p w f -> p (w f)"),
                                mybir.ActivationFunctionType.Relu)
                            dst = Z_shard[l][r0:r0 + GB * P, :].rearrange(
                                "(w p) f -> p w f", p=P)
                            nc.sync.dma_start(dst, zrow[:])
                        else:
                            for w in range(gw):
                                rows = min(P, rows_tot - w * P)
                                nc.scalar.activation(
                                    zrow[:rows, w, :], ps[:rows, w, :],
                                    mybir.ActivationFunctionType.Relu)
                                nc.sync.dma_start(
                                    Z_shard[l][r0 + w * P:r0 + w * P + rows, :],
                                    zrow[:rows, w, :])
                    nc.gpsimd.collective_compute(
                        "AllGather", mybir.AluOpType.bypass,
                        replica_groups=[list(range(NC))],
                        ins=[Z_shard[l].opt()], outs=[Z_full[l + 1].opt()])

            # ---- readout -----------------------------------------------
            gpart = res.tile([P, 1], F32)
            nc.vector.tensor_reduce(gpart[:], HT[:], mybir.AxisListType.X,
                                    mybir.AluOpType.add)
            nc.sync.dma_start(g_in[:], gpart[:])
            nc.gpsimd.collective_compute(
                "AllReduce", mybir.AluOpType.add,
                replica_groups=[list(range(NC))],
                ins=[g_in.opt()], outs=[g_out.opt()])
            gsum = res.tile([P, 1], F32)
            nc.sync.dma_start(gsum[:], g_out[:])
            gf = res.tile([P, 1], FP16)
            nc.scalar.activation(gf[:], gsum[:],
                                 mybir.ActivationFunctionType.Copy,
                                 scale=1.0 / N)
            ps1 = psp.tile([32, 1], F32, tag="head", bufs=1)
            nc.tensor.matmul(ps1[:], Wf1[:], gf[:], start=True, stop=True)
            o_col = res.tile([32, 1], FP16)
            nc.scalar.activation(o_col[:], ps1[:],
                                 mybir.ActivationFunctionType.Relu,
                                 bias=bf1[:])
            ps2 = psp.tile([1, 1], F32, tag="head", bufs=1)
            nc.tensor.matmul(ps2[:], o_col[:], Wf2[:], start=True, stop=True)
            r_sb = res.tile([1, 1], F32)
            nc.scalar.activation(r_sb[:], ps2[:],
                                 mybir.ActivationFunctionType.Sigmoid,
                                 bias=bf2[:])
            nc.sync.dma_start(out_d[:], r_sb[:])

    nc.compile()
    return nc


def make_in_maps(cfg, inputs, sched, per_core):
    x = np.asarray(inputs["x"])
    x_t = np.ascontiguousarray(x.T).astype(np.float16)
    common = dict(
        x_t=x_t,
        W1=np.asarray(inputs["W1"]).astype(np.float16),
        W2=np.asarray(inputs["W2"]).astype(np.float16),
        W3=np.asarray(inputs["W3"]).astype(np.float16),
        b1=np.asarray(inputs["b1"]).reshape(1, F).astype(np.float16),
        b2=np.asarray(inputs["b2"]).reshape(1, F).astype(np.float16),
        b3=np.asarray(inputs["b3"]).reshape(1, F).astype(np.float16),
        Wf1=np.asarray(inputs["Wf1"]).astype(np.float16),
        bf1=np.asarray(inputs["bf1"]).reshape(32, 1).astype(np.float16),
        Wf2=np.asarray(inputs["Wf2"]).astype(np.float16),
        bf2=np.asarray(inputs["bf2"]).reshape(1, 1).astype(np.float16),
        iotat=np.tile(np.arange(P, dtype=np.float16), (P, 1)),
    )
    in_maps = []
    for c in range(NC):
        m = dict(common)
        m.update(per_core[c])
        in_maps.append(m)
    return in_maps


_CACHE = {}


def kernel(x, edge_src, edge_dst, edge_weight, W1, b1, W2, b2, W3, b3,
           Wf1, bf1, Wf2, bf2):
    inputs = dict(x=x, W1=W1, b1=b1, W2=W2, b2=b2, W3=W3, b3=b3,
                  Wf1=Wf1, bf1=bf1, Wf2=Wf2, bf2=bf2)
    cfg = Cfg(N=int(np.asarray(x).shape[0]))
    sched, per_core = preprocess(cfg, edge_src, edge_dst, edge_weight)
    key = (cfg.N, sched["S"], sched["nA"], sched["nB"],
           tuple(np.asarray(sched["nsub"]).ravel().tolist()))
    if key in _CACHE:
        nc = _CACHE[key]
    else:
        nc = build(cfg, sched)
        _CACHE[key] = nc
    in_maps = make_in_maps(cfg, inputs, sched, per_core)
    res = run_bass_kernel_spmd(nc, in_maps, core_ids=list(range(NC)))
    out = np.asarray(res.results[0]["out"], dtype=np.float32)
    return out.reshape(()).astype(np.float32)



# revision 5
# speedup vs baseline: 1.6275x; 1.6275x over previous
"""Distributed GNN message-passing kernel for one TRN2 chip (8 NeuronCores).

Math (matches the reference):
    h = spmm(relu(x@W1+b1)); h = spmm(relu(h@W2+b2)); h = spmm(relu(h@W3+b3))
    g = mean(h, 0); o = relu(g@Wf1+bf1); r = sigmoid(o@Wf2+bf2)
with spmm(h)[i] = sum_{e: dst[e]=i} w[e] * h[src[e]].

Sharding: nodes block-partitioned over the 8 cores (core r owns dst rows
[r*6250, (r+1)*6250)); each edge is assigned to the core owning its dst.
Layer-1 dense is computed replicated from the full x (no comm needed);
layers 2-3 dense are computed locally on owned rows and exchanged with an
fp16 AllGather ("halo exchange" of all Z rows - the graph is uniform random
so every core needs essentially every row). The spmm itself is a per-edge
SWDGE dma_gather of Z rows (fp16, 256B descriptors, spread over the 16 DMA
engines) feeding per-128-edge one-hot segment matmuls on the TensorEngine
that accumulate H^T windows in PSUM (scatter side is free: PSUM
accumulation). Readout: local column-sum + AllReduce + tiny MLP head.

Host-side preprocessing builds the data-dependent static schedule: edges
sorted by (core, src-half, dst-window); each (half, window) group padded to
a multiple of 128 with dummy edges (idx 0, dstoff -1, w 0); subtile counts
shared across cores (max over cores) so all 8 cores run one SPMD graph.
src indices are split in two halves (< / >= 25000) because gather indices
are int16.
"""
import sys
sys.path.insert(0, '/opt/trn_rl_repo')
from dataclasses import dataclass
import numpy as np

import concourse.bass as bass
import concourse.bacc as bacc
import concourse.tile as tile
from concourse import mybir, library_config
from concourse.bass_utils import run_bass_kernel_spmd

P = 128
F = 128
NC = 8
FP16 = mybir.dt.float16
F32 = mybir.dt.float32
I16 = mybir.dt.int16


@dataclass
class Cfg:
    N: int = 50000
    CALL: int = 896  # idxs/dma_gather call: 57 descs x 2 bufs <= 128 DGE FIFO
    N_LAYERS: int = 3

    @property
    def NPC(self):
        return self.N // NC

    @property
    def NW(self):
        return (self.NPC + P - 1) // P

    @property
    def HALF(self):
        return self.N // 2

    @property
    def ROWW(self):
        return (self.N + P - 1) // P


def preprocess(cfg, edge_src, edge_dst, edge_weight):
    """Per-core gather indices / one-hot metadata + shared subtile schedule."""
    NPC, NW, HALF = cfg.NPC, cfg.NW, cfg.HALF
    edge_src = np.asarray(edge_src).astype(np.int64)
    edge_dst = np.asarray(edge_dst).astype(np.int64)
    edge_weight = np.asarray(edge_weight).astype(np.float32)
    core = edge_dst // NPC
    local = edge_dst % NPC
    win = local // P
    dstoff = local % P
    half = (edge_src >= HALF).astype(np.int64)
    srcoff = np.where(half == 1, edge_src - HALF, edge_src)

    order = np.lexsort((win, half, core))
    core_s, half_s, win_s = core[order], half[order], win[order]
    srcoff_s, dstoff_s, w_s = srcoff[order], dstoff[order], edge_weight[order]

    counts = np.zeros((NC, 2, NW), dtype=np.int64)
    np.add.at(counts, (core_s, half_s, win_s), 1)
    nsub = np.ceil(counts / P).astype(np.int64).max(axis=0)  # [2, NW]
    nsub = np.maximum(nsub, 1)

    sub_base = np.zeros((2, NW), dtype=np.int64)
    acc = 0
    for h in range(2):
        for w in range(NW):
            sub_base[h, w] = acc
            acc += nsub[h, w]
    S = acc
    nA = int(nsub[0].sum()) * P
    nB = int(nsub[1].sum()) * P

    grp_start = np.zeros(NC * 2 * NW, dtype=np.int64)
    np.cumsum(counts.ravel()[:-1], out=grp_start[1:])
    grp_start = grp_start.reshape(NC, 2, NW)

    per_core = []
    for c in range(NC):
        idx_all = np.zeros(S * P, dtype=np.int64)
        off_all = np.full(S * P, -1.0, dtype=np.float32)
        w_all = np.zeros(S * P, dtype=np.float32)
        for h in range(2):
            for w in range(NW):
                cnt = counts[c, h, w]
                g0 = grp_start[c, h, w]
                s0 = sub_base[h, w] * P
                idx_all[s0:s0 + cnt] = srcoff_s[g0:g0 + cnt]
                off_all[s0:s0 + cnt] = dstoff_s[g0:g0 + cnt]
                w_all[s0:s0 + cnt] = w_s[g0:g0 + cnt]
        idxA = idx_all[:nA].astype(np.int16)
        idxB = idx_all[nA:].astype(np.int16)
        # dma_gather idx layout: idx j -> partition j%16, col j//16,
        # replicated across the 8 gpsimd core groups
        wrapA = np.tile(idxA.reshape(-1, 16).T, (8, 1)).astype(np.int16)
        wrapB = np.tile(idxB.reshape(-1, 16).T, (8, 1)).astype(np.int16)
        off_cols = np.ascontiguousarray(off_all.reshape(S, P).T)
        w_cols = np.ascontiguousarray(w_all.reshape(S, P).T)
        per_core.append(dict(idxA=wrapA, idxB=wrapB,
                             dstoff=off_cols, wcol=w_cols))
    sched = dict(nsub=nsub, sub_base=sub_base, S=S, nA=nA, nB=nB)
    return sched, per_core


def build(cfg, sched):
    """Build the (SPMD, shared by all 8 cores) Bacc graph."""
    N, NPC, NW, HALF, ROWW, CALL = (cfg.N, cfg.NPC, cfg.NW, cfg.HALF,
                                    cfg.ROWW, cfg.CALL)
    nsub, S, nA, nB = sched["nsub"], sched["S"], sched["nA"], sched["nB"]
    nc = bacc.Bacc('TRN2', target_bir_lowering=False, debug=False,
                   num_devices=NC, num_swdge_queues=4)

    x_t = nc.dram_tensor("x_t", [P, N], FP16, kind="ExternalInput")
    idxA_d = nc.dram_tensor("idxA", [P, nA // 16], I16, kind="ExternalInput")
    idxB_d = nc.dram_tensor("idxB", [P, nB // 16], I16, kind="ExternalInput")
    dstoff_d = nc.dram_tensor("dstoff", [P, S], F32, kind="ExternalInput")
    wcol_d = nc.dram_tensor("wcol", [P, S], F32, kind="ExternalInput")
    W_d = [nc.dram_tensor(f"W{l}", [F, F], FP16, kind="ExternalInput")
           for l in (1, 2, 3)]
    b_d = [nc.dram_tensor(f"b{l}", [1, F], FP16, kind="ExternalInput")
           for l in (1, 2, 3)]
    Wf1_d = nc.dram_tensor("Wf1", [F, 32], FP16, kind="ExternalInput")
    bf1_d = nc.dram_tensor("bf1", [32, 1], FP16, kind="ExternalInput")
    Wf2_d = nc.dram_tensor("Wf2", [32, 1], FP16, kind="ExternalInput")
    bf2_d = nc.dram_tensor("bf2", [1, 1], FP16, kind="ExternalInput")
    iota_d = nc.dram_tensor("iotat", [P, P], FP16, kind="ExternalInput")
    out_d = nc.dram_tensor("out", [1, 1], F32, kind="ExternalOutput")

    with tile.TileContext(nc) as tc:
        with tc.tile_pool(name="resident", bufs=1) as res, \
             tc.tile_pool(name="xstream", bufs=4) as xs, \
             tc.tile_pool(name="stage", bufs=8) as stg, \
             tc.tile_pool(name="onehot", bufs=8) as ohp, \
             tc.tile_pool(name="zrow", bufs=4) as zrp, \
             tc.tile_pool(name="psum", bufs=2, space="PSUM") as psp, \
             tc.tile_pool(name="dram", bufs=1, space="DRAM") as drm:

            nc.gpsimd.load_library(library_config.mlp)

            idxA = res.tile([P, nA // 16], I16)
            idxB = res.tile([P, nB // 16], I16)
            dstoff = res.tile([P, S], F32)
            wcol = res.tile([P, S], F32)
            nc.sync.dma_start(idxA[:], idxA_d[:])
            nc.sync.dma_start(idxB[:], idxB_d[:])
            nc.sync.dma_start(dstoff[:], dstoff_d[:])
            nc.sync.dma_start(wcol[:], wcol_d[:])
            Ws = []
            for l in range(3):
                t = res.tile([F, F], FP16, tag=f"W{l}", name=f"Wsb{l}")
                nc.sync.dma_start(t[:], W_d[l][:])
                Ws.append(t)
            bs = []
            for l in range(3):
                t = res.tile([1, F], FP16, tag=f"b{l}", name=f"bsb{l}")
                nc.sync.dma_start(t[:], b_d[l][:])
                bs.append(t)
            Wf1 = res.tile([F, 32], FP16)
            nc.sync.dma_start(Wf1[:], Wf1_d[:])
            bf1 = res.tile([32, 1], FP16)
            nc.sync.dma_start(bf1[:], bf1_d[:])
            Wf2 = res.tile([32, 1], FP16)
            nc.sync.dma_start(Wf2[:], Wf2_d[:])
            bf2 = res.tile([1, 1], FP16)
            nc.sync.dma_start(bf2[:], bf2_d[:])
            iota = res.tile([P, P], FP16)
            nc.sync.dma_start(iota[:], iota_d[:])
            ones_row = res.tile([1, P], FP16)
            nc.vector.memset(ones_row[:], 1.0)

            # H^T accumulator for the current layer [feat, local nodes]
            HT = res.tile([P, NPC], FP16)
            self_q = [0]  # SWDGE queue round-robin counter

            # AllGather/AllReduce outputs in Shared scratchpad (peers write
            # directly); Tile tracks raw dram tensors by name.
            Z_full = [drm.tile([N, F], FP16, tag="Zfull0", name="Zfull0")]
            for l in (1, 2):
                Z_full.append(nc.dram_tensor(f"Zfull{l}", [N, F], FP16,
                                             kind="Internal",
                                             addr_space="Shared").ap())
            Z_shard = [drm.tile([NPC, F], FP16, tag=f"Zshard{l}",
                                name=f"Zshard{l}") for l in range(2)]
            g_in = drm.tile([P, 1], F32)
            g_out = nc.dram_tensor("g_out", [P, 1], F32, kind="Internal",
                                   addr_space="Shared").ap()

            # ---- layer-1 dense (replicated over all N rows) ------------
            GB = 4  # row-windows per PSUM bank / DMA batch
            for rg in range(0, ROWW, GB):
                r0 = rg * P
                gw = min(GB, ROWW - rg)
                rows_tot = min(GB * P, N - r0)
                xt_tile = xs.tile([P, GB * P], FP16, tag="xt")
                nc.sync.dma_start(xt_tile[:, :rows_tot], x_t[:, r0:r0 + rows_tot])
                ps = psp.tile([P, GB, F], F32, tag="dense", bufs=2)
                for w in range(gw):
                    rows = min(P, rows_tot - w * P)
                    nc.tensor.matmul(ps[:rows, w, :],
                                     xt_tile[:, w * P:w * P + rows], Ws[0][:],
                                     start=True, stop=False)
                    nc.tensor.matmul(ps[:rows, w, :], ones_row[:, :rows],
                                     bs[0][:], start=False, stop=True)
                zrow = zrp.tile([P, GB, F], FP16, tag="zrow")
                if rows_tot == GB * P:
                    nc.scalar.activation(
                        zrow[:].rearrange("p w f -> p (w f)"),
                        ps[:].rearrange("p w f -> p (w f)"),
                        mybir.ActivationFunctionType.Relu)
                    dst = Z_full[0][r0:r0 + GB * P, :].rearrange(
                        "(w p) f -> p w f", p=P)
                    nc.sync.dma_start(dst, zrow[:])
                else:
                    for w in range(gw):
                        rows = min(P, rows_tot - w * P)
                        nc.scalar.activation(
                            zrow[:rows, w, :], ps[:rows, w, :],
                            mybir.ActivationFunctionType.Relu)
                        nc.sync.dma_start(
                            Z_full[0][r0 + w * P:r0 + w * P + rows, :],
                            zrow[:rows, w, :])

            # ---- spmm layers -------------------------------------------
            for l in range(cfg.N_LAYERS):
                zf = Z_full[l]
                for h in range(2):
                    idx_sb = idxA if h == 0 else idxB
                    n_idx = nA if h == 0 else nB
                    src_ap = zf[h * HALF:(h + 1) * HALF, :]
                    n_sub_pass = n_idx // P
                    stages = []  # (tile, first_subtile, n_sub)
                    done = 0
                    while done < n_sub_pass:
                        k = min(CALL // P, n_sub_pass - done)
                        st = stg.tile([P, CALL // P, F], FP16, tag="gst")
                        nc.gpsimd.dma_gather(
                            out_ap=st[:, :k, :], in_ap=src_ap,
                            idxs_ap=idx_sb[:, done * P // 16:(done + k) * P // 16],
                            num_idxs=k * P, num_idxs_reg=k * P, elem_size=F,
                            queue_num=self_q[0] % 4)
                        self_q[0] += 1
                        stages.append((st, done, k))
                        done += k
                    si = 0
                    sg = 0
                    WG = 4  # windows per PSUM bank (4 x 512B = one bank)
                    for wg in range(0, NW, WG):
                        gw = min(WG, NW - wg)
                        ps = psp.tile([P, WG, P], F32, tag="spmm", bufs=3)
                        for wi in range(gw):
                            w = wg + wi
                            ns = int(nsub[h, w])
                            for k in range(ns):
                                s_glob = int(sched["sub_base"][h, w]) + k
                                st, s0, sk = stages[sg]
                                loc = si - s0
                                oh = ohp.tile([P, P], FP16, tag="oh")
                                # one-hot row e = w[e] * (iota == dstoff[e])
                                nc.vector.tensor_scalar(
                                    oh[:], iota[:],
                                    dstoff[:, s_glob:s_glob + 1],
                                    wcol[:, s_glob:s_glob + 1],
                                    mybir.AluOpType.is_equal,
                                    mybir.AluOpType.mult)
                                # H^T[:, window] += G^T(e,f) @ OH(e,seg)
                                nc.tensor.matmul(ps[:, wi, :], st[:, loc, :],
                                                 oh[:], start=(k == 0),
                                                 stop=(k == ns - 1))
                                si += 1
                                if si - s0 >= sk:
                                    sg += 1
                        c0 = wg * P
                        cols = min(WG * P, NPC - c0)
                        src = ps[:].rearrange("p w f -> p (w f)")[:, :cols]
                        if h == 0:
                            nc.vector.tensor_copy(HT[:, c0:c0 + cols], src)
                        else:
                            nc.vector.tensor_tensor(
                                HT[:, c0:c0 + cols], HT[:, c0:c0 + cols],
                                src, mybir.AluOpType.add)

                if l < cfg.N_LAYERS - 1:
                    # local dense l+2 on owned rows, then AllGather of Z
                    for rg in range(0, NW, GB):
                        r0 = rg * P
                        gw = min(GB, NW - rg)
                        rows_tot = min(GB * P, NPC - r0)
                        ps = psp.tile([P, GB, F], F32, tag="dense", bufs=2,
                                      name="ps_d2")
                        for w in range(gw):
                            rows = min(P, rows_tot - w * P)
                            nc.tensor.matmul(
                                ps[:rows, w, :],
                                HT[:, r0 + w * P:r0 + w * P + rows],
                                Ws[l + 1][:], start=True, stop=False)
                            nc.tensor.matmul(ps[:rows, w, :],
                                             ones_row[:, :rows], bs[l + 1][:],
                                             start=False, stop=True)
                        zrow = zrp.tile([P, GB, F], FP16, tag="zrow2")
                        if rows_tot == GB * P:
                            nc.scalar.activation(
                                zrow[:].rearrange("p w f -> p (w f)"),
                                ps[:].rearrange("p w f -> p (w f)"),
                                mybir.ActivationFunctionType.Relu)
                            dst = Z_shard[l][r0:r0 + GB * P, :].rearrange(
                                "(w p) f -> p w f", p=P)
                            nc.sync.dma_start(dst, zrow[:])
                        else:
                            for w in range(gw):
                                rows = min(P, rows_tot - w * P)
                                nc.scalar.activation(
                                    zrow[:rows, w, :], ps[:rows, w, :],
                                    mybir.ActivationFunctionType.Relu)
                                nc.sync.dma_start(
                                    Z_shard[l][r0 + w * P:r0 + w * P + rows, :],
                                    zrow[:rows, w, :])
                    nc.gpsimd.collective_compute(
                        "AllGather", mybir.AluOpType.bypass,
                        replica_groups=[list(range(NC))],
                        ins=[Z_shard[l].opt()], outs=[Z_full[l + 1].opt()])

            # ---- readout -----------------------------------------------
            gpart = res.tile([P, 1], F32)
            nc.vector.tensor_reduce(gpart[:], HT[:], mybir.AxisListType.X,
                                    mybir.AluOpType.add)
            nc.sync.dma_start(g_in[:], gpart[:])
            nc.gpsimd.collective_compute(
                "AllReduce", mybir.AluOpType.add,
                replica_groups=[list(range(NC))],
                ins=[g_in.opt()], outs=[g_out.opt()])
            gsum = res.tile([P, 1], F32)
            nc.sync.dma_start(gsum[:], g_out[:])
            gf = res.tile([P, 1], FP16)
            nc.scalar.activation(gf[:], gsum[:],
                                 mybir.ActivationFunctionType.Copy,
                                 scale=1.0 / N)
            ps1 = psp.tile([32, 1], F32, tag="head", bufs=1)
            nc.tensor.matmul(ps1[:], Wf1[:], gf[:], start=True, stop=True)
            o_col = res.tile([32, 1], FP16)
            nc.scalar.activation(o_col[:], ps1[:],
                                 mybir.ActivationFunctionType.Relu,
                                 bias=bf1[:])
            ps2 = psp.tile([1, 1], F32, tag="head", bufs=1)
            nc.tensor.matmul(ps2[:], o_col[:], Wf2[:], start=True, stop=True)
            r_sb = res.tile([1, 1], F32)
            nc.scalar.activation(r_sb[:], ps2[:],
                                 mybir.ActivationFunctionType.Sigmoid,
                                 bias=bf2[:])
            nc.sync.dma_start(out_d[:], r_sb[:])

    nc.compile()
    return nc


def make_in_maps(cfg, inputs, sched, per_core):
    x = np.asarray(inputs["x"])
    x_t = np.ascontiguousarray(x.T).astype(np.float16)
    common = dict(
        x_t=x_t,
        W1=np.asarray(inputs["W1"]).astype(np.float16),
        W2=np.asarray(inputs["W2"]).astype(np.float16),
        W3=np.asarray(inputs["W3"]).astype(np.float16),
        b1=np.asarray(inputs["b1"]).reshape(1, F).astype(np.float16),
        b2=np.asarray(inputs["b2"]).reshape(1, F).astype(np.float16),
        b3=np.asarray(inputs["b3"]).reshape(1, F).astype(np.float16),
        Wf1=np.asarray(inputs["Wf1"]).astype(np.float16),
        bf1=np.asarray(inputs["bf1"]).reshape(32, 1).astype(np.float16),
        Wf2=np.asarray(inputs["Wf2"]).astype(np.float16),
        bf2=np.asarray(inputs["bf2"]).reshape(1, 1).astype(np.float16),
        iotat=np.tile(np.arange(P, dtype=np.float16), (P, 1)),
    )
    in_maps = []
    for c in range(NC):
        m = dict(common)
        m.update(per_core[c])
        in_maps.append(m)
    return in_maps


_CACHE = {}


def kernel(x, edge_src, edge_dst, edge_weight, W1, b1, W2, b2, W3, b3,
           Wf1, bf1, Wf2, bf2):
    inputs = dict(x=x, W1=W1, b1=b1, W2=W2, b2=b2, W3=W3, b3=b3,
                  Wf1=Wf1, bf1=bf1, Wf2=Wf2, bf2=bf2)
    cfg = Cfg(N=int(np.asarray(x).shape[0]))
    sched, per_core = preprocess(cfg, edge_src, edge_dst, edge_weight)
    key = (cfg.N, sched["S"], sched["nA"], sched["nB"],
           tuple(np.asarray(sched["nsub"]).ravel().tolist()))
    if key in _CACHE:
        nc = _CACHE[key]
    else:
        nc = build(cfg, sched)
        _CACHE[key] = nc
    in_maps = make_in_maps(cfg, inputs, sched, per_core)
    res = run_bass_kernel_spmd(nc, in_maps, core_ids=list(range(NC)))
    out = np.asarray(res.results[0]["out"], dtype=np.float32)
    return out.reshape(()).astype(np.float32)



# revision 18
# speedup vs baseline: 1.7650x; 1.0845x over previous
"""Distributed GNN message-passing kernel for one TRN2 chip (8 NeuronCores).

Math (matches the reference):
    h = spmm(relu(x@W1+b1)); h = spmm(relu(h@W2+b2)); h = spmm(relu(h@W3+b3))
    g = mean(h, 0); o = relu(g@Wf1+bf1); r = sigmoid(o@Wf2+bf2)
with spmm(h)[i] = sum_{e: dst[e]=i} w[e] * h[src[e]].

Sharding: nodes block-partitioned over the 8 cores (core r owns dst rows
[r*6250, (r+1)*6250)); each edge is assigned to the core owning its dst.

The Z activation table for each layer is stored CHUNK-MAJOR in four separate
DRAM tensors Z_c (c = node chunk 0..3, chunk c covering local rows
[start_c, start_c+csz_c) of every core; Z_c row = r*csz_c + (i - start_c)).
This allows a) per-chunk AllGathers that overlap the spmm of later chunks,
and b) per-chunk whole-tensor dependences so layer l+1 gathers from chunk c
wait only on AllGather c.

The spmm is a per-edge SWDGE dma_gather of Z rows (fp16, 256B descriptors,
issued round-robin on the 4 SWDGE queues - queue parallelism is a ~6x
emission-throughput win on this part) feeding per-128-edge one-hot segment
matmuls on the TensorEngine that accumulate H^T windows in PSUM. Edges are
sorted by (dst core, dst chunk, src chunk, dst window) and padded to
128-multiples per (src chunk, dst window) group; all 8 cores share one
max-padded SPMD schedule. Bias adds are batched (one K=1 matmul per 4-window
PSUM group). Readout: local column-sum + AllReduce + tiny MLP head.
"""
import sys
sys.path.insert(0, '/opt/trn_rl_repo')
from dataclasses import dataclass
import numpy as np

import concourse.bass as bass
import concourse.bacc as bacc
import concourse.tile as tile
from concourse import mybir, library_config
from concourse.bass_utils import run_bass_kernel_spmd

P = 128
F = 128
NC = 8
NCHUNK = 4
DISABLE_BIAS_BATCH = False
FP16 = mybir.dt.float16
F32 = mybir.dt.float32
I16 = mybir.dt.int16


@dataclass
class Cfg:
    N: int = 50000
    CALL: int = 896  # idxs per dma_gather call (57 descs; proven FIFO-safe)
    N_LAYERS: int = 3

    @property
    def NPC(self):
        return self.N // NC

    @property
    def NW(self):
        return (self.NPC + P - 1) // P

    @property
    def WCH(self):
        """Chunk boundaries in dst windows."""
        base = self.NW // NCHUNK
        bnds = [0, base, 2 * base, 3 * base, self.NW]
        return [(bnds[i], bnds[i + 1]) for i in range(NCHUNK)]

    @property
    def CSZ(self):
        """Rows per chunk (last chunk ragged)."""
        out = []
        for (w0, w1) in self.WCH:
            out.append(min(w1 * P, self.NPC) - w0 * P)
        return out

    @property
    def CSTART(self):
        return [w0 * P for (w0, w1) in self.WCH]


def preprocess(cfg, edge_src, edge_dst, edge_weight):
    """Edge schedule: sort by (dst core, dst chunk, src chunk, dst window);
    pad each (src chunk, dst window) group to a multiple of 128 (shared
    max-over-cores subtile counts so all 8 cores run one SPMD graph)."""
    NPC, NW = cfg.NPC, cfg.NW
    CSZ, CSTART = cfg.CSZ, cfg.CSTART
    edge_src = np.asarray(edge_src).astype(np.int64)
    edge_dst = np.asarray(edge_dst).astype(np.int64)
    edge_weight = np.asarray(edge_weight).astype(np.float32)

    core = edge_dst // NPC
    local = edge_dst % NPC
    win = local // P
    dstoff = local % P
    # chunk of a local row index
    cbnd = np.asarray(CSTART + [NPC])

    src_r = edge_src // NPC
    src_i = edge_src % NPC
    srcq = np.searchsorted(cbnd, src_i, side='right') - 1  # src chunk 0..3
    csz_arr = np.asarray(CSZ)
    cst_arr = np.asarray(CSTART)
    srcoff = src_r * csz_arr[srcq] + (src_i - cst_arr[srcq])  # row in Z_c

    order = np.lexsort((win, srcq, core))
    core_s = core[order]
    srcq_s = srcq[order]
    win_s = win[order]
    srcoff_s = srcoff[order]
    dstoff_s = dstoff[order]
    w_s = edge_weight[order]

    counts = np.zeros((NC, NCHUNK, NW), dtype=np.int64)
    np.add.at(counts, (core_s, srcq_s, win_s), 1)
    nsub = np.ceil(counts / P).astype(np.int64).max(axis=0)  # [NCHUNK, NW]
    nsub = np.maximum(nsub, 1)

    # flat subtile order: for cd: for srcq: for w in chunk cd
    sub_base = np.zeros((NCHUNK, NW), dtype=np.int64)
    acc = 0
    for ci, (w0, w1) in enumerate(cfg.WCH):
        for q in range(NCHUNK):
            for w in range(w0, w1):
                sub_base[q, w] = acc
                acc += nsub[q, w]
    S = acc

    grp_start = np.zeros(NC * NCHUNK * NW, dtype=np.int64)
    np.cumsum(counts.ravel()[:-1], out=grp_start[1:])
    grp_start = grp_start.reshape(NC, NCHUNK, NW)

    per_core = []
    for c in range(NC):
        idx_all = np.zeros(S * P, dtype=np.int64)
        off_all = np.full(S * P, -1.0, dtype=np.float32)
        w_all = np.zeros(S * P, dtype=np.float32)
        for q in range(NCHUNK):
            for w in range(NW):
                cnt = counts[c, q, w]
                g0 = grp_start[c, q, w]
                s0 = sub_base[q, w] * P
                idx_all[s0:s0 + cnt] = srcoff_s[g0:g0 + cnt]
                off_all[s0:s0 + cnt] = dstoff_s[g0:g0 + cnt]
                w_all[s0:s0 + cnt] = w_s[g0:g0 + cnt]
        idx16 = idx_all.astype(np.int16)
        # dma_gather idx layout: idx j -> partition j%16, col j//16,
        # replicated across the 8 gpsimd core groups
        wrap = np.tile(idx16.reshape(-1, 16).T, (8, 1)).astype(np.int16)
        off_cols = np.ascontiguousarray(off_all.reshape(S, P).T)
        w_cols = np.ascontiguousarray(w_all.reshape(S, P).T)
        per_core.append(dict(idx=wrap, dstoff=off_cols, wcol=w_cols))
    sched = dict(nsub=nsub, sub_base=sub_base, S=S)
    return sched, per_core


def build(cfg, sched, dump_ht=False):
    """Build the (SPMD, shared by all 8 cores) Bacc graph."""
    N, NPC, NW, CALL = cfg.N, cfg.NPC, cfg.NW, cfg.CALL
    WCH, CSZ, CSTART = cfg.WCH, cfg.CSZ, cfg.CSTART
    nsub, S = sched["nsub"], sched["S"]
    nc = bacc.Bacc('TRN2', target_bir_lowering=False, debug=False,
                   num_devices=NC, num_swdge_queues=4)

    x_t = nc.dram_tensor("x_t", [P, N], FP16, kind="ExternalInput")
    idx_d = nc.dram_tensor("idx", [P, S * 8], I16, kind="ExternalInput")
    dstoff_d = nc.dram_tensor("dstoff", [P, S], F32, kind="ExternalInput")
    wcol_d = nc.dram_tensor("wcol", [P, S], F32, kind="ExternalInput")
    W_d = [nc.dram_tensor(f"W{l}", [F, F], FP16, kind="ExternalInput")
           for l in (1, 2, 3)]
    b4_d = [nc.dram_tensor(f"b4_{l}", [1, 4 * F], FP16, kind="ExternalInput")
            for l in (1, 2, 3)]
    Wf1_d = nc.dram_tensor("Wf1", [F, 32], FP16, kind="ExternalInput")
    bf1_d = nc.dram_tensor("bf1", [32, 1], FP16, kind="ExternalInput")
    Wf2_d = nc.dram_tensor("Wf2", [32, 1], FP16, kind="ExternalInput")
    bf2_d = nc.dram_tensor("bf2", [1, 1], FP16, kind="ExternalInput")
    iota_d = nc.dram_tensor("iotat", [P, P], FP16, kind="ExternalInput")
    out_d = nc.dram_tensor("out", [1, 1], F32, kind="ExternalOutput")
    ht_out_d = (nc.dram_tensor("HT_out", [P, cfg.NPC], F32,
                               kind="ExternalOutput") if dump_ht else None)
    z0_out_d = (nc.dram_tensor("Z0_out", [NC * CSZ[0], F], FP16,
                               kind="ExternalOutput") if dump_ht else None)

    with tile.TileContext(nc) as tc:
        with tc.tile_pool(name="resident", bufs=1) as res, \
             tc.tile_pool(name="xstream", bufs=4) as xs, \
             tc.tile_pool(name="stage", bufs=8) as stg, \
             tc.tile_pool(name="onehot", bufs=8) as ohp, \
             tc.tile_pool(name="zrow", bufs=4) as zrp, \
             tc.tile_pool(name="psum", bufs=2, space="PSUM") as psp, \
             tc.tile_pool(name="dram", bufs=1, space="DRAM") as drm:

            nc.gpsimd.load_library(library_config.mlp)

            idx = res.tile([P, S * 8], I16)
            dstoff = res.tile([P, S], F32)
            wcol = res.tile([P, S], F32)
            nc.sync.dma_start(idx[:], idx_d[:])
            nc.sync.dma_start(dstoff[:], dstoff_d[:])
            nc.sync.dma_start(wcol[:], wcol_d[:])
            Ws = []
            for l in range(3):
                t = res.tile([F, F], FP16, tag=f"W{l}", name=f"Wsb{l}")
                nc.sync.dma_start(t[:], W_d[l][:])
                Ws.append(t)
            b4s = []
            for l in range(3):
                t = res.tile([1, 4 * F], FP16, tag=f"b4_{l}", name=f"b4sb{l}")
                nc.sync.dma_start(t[:], b4_d[l][:])
                b4s.append(t)
            Wf1 = res.tile([F, 32], FP16)
            nc.sync.dma_start(Wf1[:], Wf1_d[:])
            bf1 = res.tile([32, 1], FP16)
            nc.sync.dma_start(bf1[:], bf1_d[:])
            Wf2 = res.tile([32, 1], FP16)
            nc.sync.dma_start(Wf2[:], Wf2_d[:])
            bf2 = res.tile([1, 1], FP16)
            nc.sync.dma_start(bf2[:], bf2_d[:])
            iota = res.tile([P, P], FP16)
            nc.sync.dma_start(iota[:], iota_d[:])
            ones_row = res.tile([1, P], FP16)
            nc.vector.memset(ones_row[:], 1.0)

            # H^T accumulator for the current layer [feat, local nodes]
            HT = res.tile([P, NPC], FP16)

            # chunked Z tables: Z[l][c] has 8*CSZ[c] rows (all cores' chunk c)
            Z = []
            for l in range(cfg.N_LAYERS):
                row = []
                for c in range(NCHUNK):
                    if l == 0:
                        row.append(drm.tile([NC * CSZ[c], F], FP16,
                                            tag=f"Z0_{c}", name=f"Z0_{c}"))
                    else:
                        row.append(nc.dram_tensor(
                            f"Z{l}_{c}", [NC * CSZ[c], F], FP16,
                            kind="Internal", addr_space="Shared").ap())
                Z.append(row)
            # AllGather input staging (this core's dense output, per chunk)
            Zsh = [[drm.tile([CSZ[c], F], FP16, tag=f"Zsh{l}_{c}",
                            name=f"Zsh{l}_{c}") for c in range(NCHUNK)]
                   for l in range(2)]
            g_in = drm.tile([P, 1], F32)
            g_out = nc.dram_tensor("g_out", [P, 1], F32, kind="Internal",
                                   addr_space="Shared").ap()

            qrr = [0]  # SWDGE queue round-robin

            def dense_group(ps, src_tiles, Wl, b4, nwin, rows_tot):
                """matmuls for one <=4-window dense group into psum tile ps
                [P, 4, F]; src_tiles(wloc, rows) -> lhsT AP [f, rows].
                Per-window (data start + bias stop) pairs, same as the
                proven baseline shape."""
                for wl in range(nwin):
                    rows = min(P, rows_tot - wl * P)
                    nc.tensor.matmul(ps[:rows, wl, :], src_tiles(wl, rows),
                                     Wl[:], start=True, stop=False)
                    nc.tensor.matmul(ps[:rows, wl, :], ones_row[:, :rows],
                                     b4[:, wl * F:wl * F + F],
                                     start=False, stop=True)

            def emit_dense_chunk(l, c):
                """Dense layer l+2 on HT cols of chunk c -> Zsh, then chunked
                AllGather into Z[l+1][c]."""
                csz, cst = CSZ[c], CSTART[c]
                for g0 in range(0, csz, 4 * P):
                    rows_tot = min(4 * P, csz - g0)
                    nwin = (rows_tot + P - 1) // P
                    ps = psp.tile([P, 4, F], F32, tag="dense", bufs=2,
                                  name="ps_d2")
                    dense_group(
                        ps,
                        lambda wl, rows: HT[:, cst + g0 + wl * P:
                                            cst + g0 + wl * P + rows],
                        Ws[l + 1], b4s[l + 1], nwin, rows_tot)
                    zrow = zrp.tile([P, 4, F], FP16, tag="zrow2")
                    if rows_tot == 4 * P:
                        nc.scalar.activation(
                            zrow[:].rearrange("p w f -> p (w f)"),
                            ps[:].rearrange("p w f -> p (w f)"),
                            mybir.ActivationFunctionType.Relu)
                        dst = Zsh[l][c][g0:g0 + 4 * P, :].rearrange(
                            "(w p) f -> p w f", p=P)
                        nc.sync.dma_start(dst, zrow[:])
                    else:
                        for wl in range(nwin):
                            rows = min(P, rows_tot - wl * P)
                            nc.scalar.activation(
                                zrow[:rows, wl, :], ps[:rows, wl, :],
                                mybir.ActivationFunctionType.Relu)
                            nc.sync.dma_start(
                                Zsh[l][c][g0 + wl * P:g0 + wl * P + rows, :],
                                zrow[:rows, wl, :])
                nc.gpsimd.collective_compute(
                    "AllGather", mybir.AluOpType.bypass,
                    replica_groups=[list(range(NC))],
                    ins=[Zsh[l][c].opt()], outs=[Z[l + 1][c].opt()])

            # ---- layer-1 dense (replicated; chunk-major output) ---------
            for c in range(NCHUNK):
                csz, cst = CSZ[c], CSTART[c]
                for r in range(NC):
                    col0 = r * NPC + cst
                    out0 = r * csz
                    for g0 in range(0, csz, 4 * P):
                        rows_tot = min(4 * P, csz - g0)
                        nwin = (rows_tot + P - 1) // P
                        xt_tile = xs.tile([P, 4 * P], FP16, tag="xt")
                        nc.sync.dma_start(xt_tile[:, :rows_tot],
                                          x_t[:, col0 + g0:col0 + g0 + rows_tot])
                        ps = psp.tile([P, 4, F], F32, tag="dense", bufs=2)
                        dense_group(
                            ps,
                            lambda wl, rows: xt_tile[:, wl * P:wl * P + rows],
                            Ws[0], b4s[0], nwin, rows_tot)
                        zrow = zrp.tile([P, 4, F], FP16, tag="zrow")
                        if rows_tot == 4 * P:
                            nc.scalar.activation(
                                zrow[:].rearrange("p w f -> p (w f)"),
                                ps[:].rearrange("p w f -> p (w f)"),
                                mybir.ActivationFunctionType.Relu)
                            dst = Z[0][c][out0 + g0:out0 + g0 + 4 * P, :]\
                                .rearrange("(w p) f -> p w f", p=P)
                            nc.sync.dma_start(dst, zrow[:])
                        else:
                            for wl in range(nwin):
                                rows = min(P, rows_tot - wl * P)
                                nc.scalar.activation(
                                    zrow[:rows, wl, :], ps[:rows, wl, :],
                                    mybir.ActivationFunctionType.Relu)
                                nc.sync.dma_start(
                                    Z[0][c][out0 + g0 + wl * P:
                                            out0 + g0 + wl * P + rows, :],
                                    zrow[:rows, wl, :])

            # ---- spmm layers --------------------------------------------
            for l in range(cfg.N_LAYERS):
                si = 0
                for cd, (w0, w1) in enumerate(WCH):
                    # gather runs for this chunk: one per src chunk
                    stages = []  # (tile, first_subtile, n_sub)
                    for q in range(NCHUNK):
                        run = int(nsub[q, w0:w1].sum())
                        src_ap = Z[l][q][:]
                        done = si
                        end = si + run
                        while done < end:
                            k = min(CALL // P, end - done)
                            st = stg.tile([P, CALL // P, F], FP16, tag="gst")
                            nc.gpsimd.dma_gather(
                                out_ap=st[:, :k, :], in_ap=src_ap,
                                idxs_ap=idx[:, done * 8:(done + k) * 8],
                                num_idxs=k * P, num_idxs_reg=k * P,
                                elem_size=F, queue_num=qrr[0] % 4)
                            qrr[0] += 1
                            stages.append((st, done, k))
                            done += k
                        si = end

                    # overlap previous chunk's dense+AllGather with this
                    # chunk's gathers (emitted after them on the Pool queue)
                    if l < cfg.N_LAYERS - 1 and cd >= 1:
                        emit_dense_chunk(l, cd - 1)

                    # consumption: one clean PSUM accumulation pass per src
                    # chunk; DVE copy/add folds the 4 passes into HT
                    nwinc = w1 - w0
                    groups = [(g, min(g + 4, nwinc))
                              for g in range(0, nwinc, 4)]
                    sg = 0
                    sii = stages[0][1]
                    for q in range(NCHUNK):
                        pst = [psp.tile([P, gw1 - gw0, P], F32, tag="spmm",
                                        bufs=5, name="ps_spmm")
                               for (gw0, gw1) in groups]
                        for w in range(w0, w1):
                            gi = (w - w0) // 4
                            wi = (w - w0) % 4
                            ns = int(nsub[q, w])
                            for k in range(ns):
                                st, s0, sk = stages[sg]
                                loc = sii - s0
                                oh = ohp.tile([P, P], FP16, tag="oh")
                                # one-hot row e = w[e] * (iota == dstoff[e])
                                nc.vector.tensor_scalar(
                                    oh[:], iota[:],
                                    dstoff[:, sii:sii + 1],
                                    wcol[:, sii:sii + 1],
                                    mybir.AluOpType.is_equal,
                                    mybir.AluOpType.mult)
                                # H^T[:, w] += G^T(e,f) @ OH(e,seg)
                                nc.tensor.matmul(
                                    pst[gi][:, wi, :], st[:, loc, :], oh[:],
                                    start=(k == 0), stop=(k == ns - 1))
                                sii += 1
                                if sii - s0 >= sk:
                                    sg += 1
                        for (gw0, gw1), ps in zip(groups, pst):
                            c0 = CSTART[cd] + gw0 * P
                            cols = min((gw1 - gw0) * P, NPC - c0)
                            src = ps[:].rearrange("p w f -> p (w f)")[:, :cols]
                            if q == 0:
                                nc.vector.tensor_copy(HT[:, c0:c0 + cols], src)
                            else:
                                nc.vector.tensor_tensor(
                                    HT[:, c0:c0 + cols], HT[:, c0:c0 + cols],
                                    src, mybir.AluOpType.add)

                if l < cfg.N_LAYERS - 1:
                    emit_dense_chunk(l, NCHUNK - 1)

            # ---- readout -----------------------------------------------
            if dump_ht:
                htf = res.tile([P, NPC], F32, name="htf")
                nc.vector.tensor_copy(htf[:], HT[:])
                nc.sync.dma_start(ht_out_d[:], htf[:])
                nrow0 = NC * CSZ[0]
                for r0 in range(0, nrow0, P):
                    rr = min(P, nrow0 - r0)
                    zt = zrp.tile([P, F], FP16, tag="z0dump", name="z0dump")
                    nc.sync.dma_start(zt[:rr, :], Z[0][0][r0:r0 + rr, :])
                    nc.sync.dma_start(z0_out_d[r0:r0 + rr, :], zt[:rr, :])
            gpart = res.tile([P, 1], F32)
            nc.vector.tensor_reduce(gpart[:], HT[:], mybir.AxisListType.X,
                                    mybir.AluOpType.add)
            nc.sync.dma_start(g_in[:], gpart[:])
            nc.gpsimd.collective_compute(
                "AllReduce", mybir.AluOpType.add,
                replica_groups=[list(range(NC))],
                ins=[g_in.opt()], outs=[g_out.opt()])
            gsum = res.tile([P, 1], F32)
            nc.sync.dma_start(gsum[:], g_out[:])
            gf = res.tile([P, 1], FP16)
            nc.scalar.activation(gf[:], gsum[:],
                                 mybir.ActivationFunctionType.Copy,
                                 scale=1.0 / N)
            ps1 = psp.tile([32, 1], F32, tag="head", bufs=1)
            nc.tensor.matmul(ps1[:], Wf1[:], gf[:], start=True, stop=True)
            o_col = res.tile([32, 1], FP16)
            nc.scalar.activation(o_col[:], ps1[:],
                                 mybir.ActivationFunctionType.Relu,
                                 bias=bf1[:])
            ps2 = psp.tile([1, 1], F32, tag="head", bufs=1)
            nc.tensor.matmul(ps2[:], o_col[:], Wf2[:], start=True, stop=True)
            r_sb = res.tile([1, 1], F32)
            nc.scalar.activation(r_sb[:], ps2[:],
                                 mybir.ActivationFunctionType.Sigmoid,
                                 bias=bf2[:])
            nc.sync.dma_start(out_d[:], r_sb[:])

    nc.compile()
    return nc


def make_in_maps(cfg, inputs, sched, per_core):
    x = np.asarray(inputs["x"])
    x_t = np.ascontiguousarray(x.T).astype(np.float16)

    def b4(b):
        return np.tile(np.asarray(b).reshape(1, F), (1, 4)).astype(np.float16)

    common = dict(
        x_t=x_t,
        W1=np.asarray(inputs["W1"]).astype(np.float16),
        W2=np.asarray(inputs["W2"]).astype(np.float16),
        W3=np.asarray(inputs["W3"]).astype(np.float16),
        b4_1=b4(inputs["b1"]),
        b4_2=b4(inputs["b2"]),
        b4_3=b4(inputs["b3"]),
        Wf1=np.asarray(inputs["Wf1"]).astype(np.float16),
        bf1=np.asarray(inputs["bf1"]).reshape(32, 1).astype(np.float16),
        Wf2=np.asarray(inputs["Wf2"]).astype(np.float16),
        bf2=np.asarray(inputs["bf2"]).reshape(1, 1).astype(np.float16),
        iotat=np.tile(np.arange(P, dtype=np.float16), (P, 1)),
    )
    in_maps = []
    for c in range(NC):
        m = dict(common)
        m.update(per_core[c])
        in_maps.append(m)
    return in_maps


_CACHE = {}


def kernel(x, edge_src, edge_dst, edge_weight, W1, b1, W2, b2, W3, b3,
           Wf1, bf1, Wf2, bf2):
    inputs = dict(x=x, W1=W1, b1=b1, W2=W2, b2=b2, W3=W3, b3=b3,
                  Wf1=Wf1, bf1=bf1, Wf2=Wf2, bf2=bf2)
    cfg = Cfg(N=int(np.asarray(x).shape[0]))
    sched, per_core = preprocess(cfg, edge_src, edge_dst, edge_weight)
    key = (cfg.N, sched["S"],
           tuple(np.asarray(sched["nsub"]).ravel().tolist()))
    if key in _CACHE:
        nc = _CACHE[key]
    else:
        nc = build(cfg, sched)
        _CACHE[key] = nc
    in_maps = make_in_maps(cfg, inputs, sched, per_core)
    res = run_bass_kernel_spmd(nc, in_maps, core_ids=list(range(NC)))
    out = np.asarray(res.results[0]["out"], dtype=np.float32)
    return out.reshape(()).astype(np.float32)


# revision 33
# speedup vs baseline: 1.8237x; 1.0333x over previous
"""Distributed GNN message-passing kernel for one TRN2 chip (8 NeuronCores).

Math (matches the reference):
    h = spmm(relu(x@W1+b1)); h = spmm(relu(h@W2+b2)); h = spmm(relu(h@W3+b3))
    g = mean(h, 0); o = relu(g@Wf1+bf1); r = sigmoid(o@Wf2+bf2)
with spmm(h)[i] = sum_{e: dst[e]=i} w[e] * h[src[e]].

Sharding: nodes block-partitioned over the 8 cores (core r owns dst rows
[r*6250, (r+1)*6250)); each edge is assigned to the core owning its dst.

The Z activation table for each layer is stored CHUNK-MAJOR in four separate
DRAM tensors Z_c (c = node chunk 0..3, chunk c covering local rows
[start_c, start_c+csz_c) of every core; Z_c row = r*csz_c + (i - start_c)).
This allows a) per-chunk AllGathers that overlap the spmm of later chunks,
and b) per-chunk whole-tensor dependences so layer l+1 gathers from chunk c
wait only on AllGather c.

The spmm is a per-edge SWDGE dma_gather of Z rows (fp16, 256B descriptors,
issued round-robin on the 4 SWDGE queues - queue parallelism is a ~6x
emission-throughput win on this part) feeding per-128-edge one-hot segment
matmuls on the TensorEngine that accumulate H^T windows in PSUM. Edges are
sorted by (dst core, dst chunk, src chunk, dst window) and padded to
128-multiples per (src chunk, dst window) group; all 8 cores share one
max-padded SPMD schedule. Bias adds are batched (one K=1 matmul per 4-window
PSUM group). Readout: local column-sum + AllReduce + tiny MLP head.
"""
import sys
sys.path.insert(0, '/opt/trn_rl_repo')
from dataclasses import dataclass
import numpy as np

import concourse.bass as bass
import concourse.bacc as bacc
import concourse.tile as tile
from concourse import mybir, library_config
from concourse.bass_utils import run_bass_kernel_spmd

P = 128
F = 128
NC = 8
NCHUNK = 4
DISABLE_BIAS_BATCH = False
ABLATE = None  # None | 'no_oh' | 'no_mm'
FP16 = mybir.dt.float16
F32 = mybir.dt.float32
I16 = mybir.dt.int16


@dataclass
class Cfg:
    N: int = 50000
    CALL: int = 896  # idxs per dma_gather call (57 descs; proven FIFO-safe)
    N_LAYERS: int = 3

    @property
    def NPC(self):
        return self.N // NC

    @property
    def NW(self):
        return (self.NPC + P - 1) // P

    @property
    def WCH(self):
        """Chunk boundaries in dst windows."""
        base = self.NW // NCHUNK
        bnds = [0, base, 2 * base, 3 * base, self.NW]
        return [(bnds[i], bnds[i + 1]) for i in range(NCHUNK)]

    @property
    def CSZ(self):
        """Rows per chunk (last chunk ragged)."""
        out = []
        for (w0, w1) in self.WCH:
            out.append(min(w1 * P, self.NPC) - w0 * P)
        return out

    @property
    def CSTART(self):
        return [w0 * P for (w0, w1) in self.WCH]


def preprocess(cfg, edge_src, edge_dst, edge_weight):
    """Edge schedule: sort by (dst core, dst chunk, src chunk, dst window);
    pad each (src chunk, dst window) group to a multiple of 128 (shared
    max-over-cores subtile counts so all 8 cores run one SPMD graph)."""
    NPC, NW = cfg.NPC, cfg.NW
    CSZ, CSTART = cfg.CSZ, cfg.CSTART
    edge_src = np.asarray(edge_src).astype(np.int64)
    edge_dst = np.asarray(edge_dst).astype(np.int64)
    edge_weight = np.asarray(edge_weight).astype(np.float32)

    core = edge_dst // NPC
    local = edge_dst % NPC
    win = local // P
    dstoff = local % P
    # chunk of a local row index
    cbnd = np.asarray(CSTART + [NPC])

    src_r = edge_src // NPC
    src_i = edge_src % NPC
    srcq = np.searchsorted(cbnd, src_i, side='right') - 1  # src chunk 0..3
    csz_arr = np.asarray(CSZ)
    cst_arr = np.asarray(CSTART)
    srcoff = src_r * csz_arr[srcq] + (src_i - cst_arr[srcq])  # row in Z_c

    order = np.lexsort((win, srcq, core))
    core_s = core[order]
    srcq_s = srcq[order]
    win_s = win[order]
    srcoff_s = srcoff[order]
    dstoff_s = dstoff[order]
    w_s = edge_weight[order]

    counts = np.zeros((NC, NCHUNK, NW), dtype=np.int64)
    np.add.at(counts, (core_s, srcq_s, win_s), 1)
    nsub = np.ceil(counts / P).astype(np.int64).max(axis=0)  # [NCHUNK, NW]
    nsub = np.maximum(nsub, 1)

    # flat subtile order: for cd: for srcq: for w in chunk cd
    sub_base = np.zeros((NCHUNK, NW), dtype=np.int64)
    acc = 0
    for ci, (w0, w1) in enumerate(cfg.WCH):
        for q in range(NCHUNK):
            for w in range(w0, w1):
                sub_base[q, w] = acc
                acc += nsub[q, w]
    S = acc

    grp_start = np.zeros(NC * NCHUNK * NW, dtype=np.int64)
    np.cumsum(counts.ravel()[:-1], out=grp_start[1:])
    grp_start = grp_start.reshape(NC, NCHUNK, NW)

    per_core = []
    for c in range(NC):
        idx_all = np.zeros(S * P, dtype=np.int64)
        off_all = np.full(S * P, -1.0, dtype=np.float32)
        w_all = np.zeros(S * P, dtype=np.float32)
        for q in range(NCHUNK):
            for w in range(NW):
                cnt = counts[c, q, w]
                g0 = grp_start[c, q, w]
                s0 = sub_base[q, w] * P
                idx_all[s0:s0 + cnt] = srcoff_s[g0:g0 + cnt]
                off_all[s0:s0 + cnt] = dstoff_s[g0:g0 + cnt]
                w_all[s0:s0 + cnt] = w_s[g0:g0 + cnt]
        idx16 = idx_all.astype(np.int16)
        # dma_gather idx layout: idx j -> partition j%16, col j//16,
        # replicated across the 8 gpsimd core groups
        wrap = np.tile(idx16.reshape(-1, 16).T, (8, 1)).astype(np.int16)
        off_cols = np.ascontiguousarray(off_all.reshape(S, P).T)
        w_cols = np.ascontiguousarray(w_all.reshape(S, P).T)
        per_core.append(dict(idx=wrap, dstoff=off_cols, wcol=w_cols))
    sched = dict(nsub=nsub, sub_base=sub_base, S=S)
    return sched, per_core


def build(cfg, sched, dump_ht=False):
    """Build the (SPMD, shared by all 8 cores) Bacc graph."""
    N, NPC, NW, CALL = cfg.N, cfg.NPC, cfg.NW, cfg.CALL
    WCH, CSZ, CSTART = cfg.WCH, cfg.CSZ, cfg.CSTART
    nsub, S = sched["nsub"], sched["S"]
    nc = bacc.Bacc('TRN2', target_bir_lowering=False, debug=False,
                   num_devices=NC, num_swdge_queues=4)

    x_t = nc.dram_tensor("x_t", [P, N], FP16, kind="ExternalInput")
    idx_d = nc.dram_tensor("idx", [P, S * 8], I16, kind="ExternalInput")
    dstoff_d = nc.dram_tensor("dstoff", [P, S], F32, kind="ExternalInput")
    wcol_d = nc.dram_tensor("wcol", [P, S], F32, kind="ExternalInput")
    W_d = [nc.dram_tensor(f"W{l}", [F, F], FP16, kind="ExternalInput")
           for l in (1, 2, 3)]
    b4_d = [nc.dram_tensor(f"b4_{l}", [1, 4 * F], FP16, kind="ExternalInput")
            for l in (1, 2, 3)]
    Wf1_d = nc.dram_tensor("Wf1", [F, 32], FP16, kind="ExternalInput")
    bf1_d = nc.dram_tensor("bf1", [32, 1], FP16, kind="ExternalInput")
    Wf2_d = nc.dram_tensor("Wf2", [32, 1], FP16, kind="ExternalInput")
    bf2_d = nc.dram_tensor("bf2", [1, 1], FP16, kind="ExternalInput")
    iota_d = nc.dram_tensor("iotat", [P, P], FP16, kind="ExternalInput")
    out_d = nc.dram_tensor("out", [1, 1], F32, kind="ExternalOutput")
    ht_out_d = (nc.dram_tensor("HT_out", [P, cfg.NPC], F32,
                               kind="ExternalOutput") if dump_ht else None)
    z0_out_d = (nc.dram_tensor("Z0_out", [NC * CSZ[0], F], FP16,
                               kind="ExternalOutput") if dump_ht else None)

    with tile.TileContext(nc) as tc:
        with tc.tile_pool(name="resident", bufs=1) as res, \
             tc.tile_pool(name="xstream", bufs=4) as xs, \
             tc.tile_pool(name="stage", bufs=8) as stg, \
             tc.tile_pool(name="onehot", bufs=24) as ohp, \
             tc.tile_pool(name="zrow", bufs=4) as zrp, \
             tc.tile_pool(name="psum", bufs=2, space="PSUM") as psp, \
             tc.tile_pool(name="dram", bufs=1, space="DRAM") as drm:

            nc.gpsimd.load_library(library_config.mlp)

            idx = res.tile([P, S * 8], I16)
            dstoff = res.tile([P, S], F32)
            wcol = res.tile([P, S], F32)
            nc.sync.dma_start(idx[:], idx_d[:])
            nc.sync.dma_start(dstoff[:], dstoff_d[:])
            nc.sync.dma_start(wcol[:], wcol_d[:])
            Ws = []
            for l in range(3):
                t = res.tile([F, F], FP16, tag=f"W{l}", name=f"Wsb{l}")
                nc.sync.dma_start(t[:], W_d[l][:])
                Ws.append(t)
            b4s = []
            for l in range(3):
                t = res.tile([1, 4 * F], FP16, tag=f"b4_{l}", name=f"b4sb{l}")
                nc.sync.dma_start(t[:], b4_d[l][:])
                b4s.append(t)
            Wf1 = res.tile([F, 32], FP16)
            nc.sync.dma_start(Wf1[:], Wf1_d[:])
            bf1 = res.tile([32, 1], FP16)
            nc.sync.dma_start(bf1[:], bf1_d[:])
            Wf2 = res.tile([32, 1], FP16)
            nc.sync.dma_start(Wf2[:], Wf2_d[:])
            bf2 = res.tile([1, 1], FP16)
            nc.sync.dma_start(bf2[:], bf2_d[:])
            iota = res.tile([P, P], FP16)
            nc.sync.dma_start(iota[:], iota_d[:])
            ones_row = res.tile([1, P], FP16)
            nc.vector.memset(ones_row[:], 1.0)
            oh_dummy = None
            if ABLATE == 'no_oh':
                oh_dummy = res.tile([P, P], FP16, name="ohdummy")
                nc.vector.memset(oh_dummy[:], 0.0)

            # H^T accumulators, one per src-chunk pass [feat, local nodes];
            # the dense layers / readout fold the 4 partials (keeps all
            # PSUM->SBUF copies on the idle ACT engine, no DVE adds)
            HT4 = [res.tile([P, NPC], FP16, name=f"HTq{q}")
                   for q in range(NCHUNK)]
            if ABLATE == 'no_mm':
                for q in range(NCHUNK):
                    nc.vector.memset(HT4[q][:], 0.0)

            # chunked Z tables: Z[l][c] has 8*CSZ[c] rows (all cores' chunk c)
            Z = []
            for l in range(cfg.N_LAYERS):
                row = []
                for c in range(NCHUNK):
                    if l == 0:
                        row.append(drm.tile([NC * CSZ[c], F], FP16,
                                            tag=f"Z0_{c}", name=f"Z0_{c}"))
                    else:
                        row.append(nc.dram_tensor(
                            f"Z{l}_{c}", [NC * CSZ[c], F], FP16,
                            kind="Internal", addr_space="Shared").ap())
                Z.append(row)
            # AllGather input staging (this core's dense output, per chunk)
            Zsh = [[drm.tile([CSZ[c], F], FP16, tag=f"Zsh{l}_{c}",
                            name=f"Zsh{l}_{c}") for c in range(NCHUNK)]
                   for l in range(2)]
            g_in = drm.tile([P, 1], F32)
            g_out = nc.dram_tensor("g_out", [P, 1], F32, kind="Internal",
                                   addr_space="Shared").ap()

            qrr = [0]  # SWDGE queue round-robin

            def dense_group(ps, src_fns, Wl, b4, nwin, rows_tot):
                """matmuls for one <=4-window dense group into psum tile ps
                [P, 4, F]; src_fns: list of (wloc, rows) -> lhsT AP [f, rows]
                whose contributions accumulate per window (first opens the
                has_written group, the bias matmul closes it)."""
                for wl in range(nwin):
                    rows = min(P, rows_tot - wl * P)
                    for i, fn in enumerate(src_fns):
                        nc.tensor.matmul(ps[:rows, wl, :], fn(wl, rows),
                                         Wl[:], start=(i == 0), stop=False)
                    nc.tensor.matmul(ps[:rows, wl, :], ones_row[:, :rows],
                                     b4[:, wl * F:wl * F + F],
                                     start=False, stop=True)

            def emit_dense_chunk(l, c):
                """Dense layer l+2 on HT cols of chunk c -> Zsh, then chunked
                AllGather into Z[l+1][c]."""
                csz, cst = CSZ[c], CSTART[c]
                for g0 in range(0, csz, 4 * P):
                    rows_tot = min(4 * P, csz - g0)
                    nwin = (rows_tot + P - 1) // P
                    ps = psp.tile([P, 4, F], F32, tag="dense", bufs=2,
                                  name="ps_d2")
                    dense_group(
                        ps,
                        [(lambda wl, rows, q=q:
                          HT4[q][:, cst + g0 + wl * P:
                                 cst + g0 + wl * P + rows])
                         for q in range(NCHUNK)],
                        Ws[l + 1], b4s[l + 1], nwin, rows_tot)
                    zrow = zrp.tile([P, 4, F], FP16, tag="zrow2")
                    if rows_tot == 4 * P:
                        nc.scalar.activation(
                            zrow[:].rearrange("p w f -> p (w f)"),
                            ps[:].rearrange("p w f -> p (w f)"),
                            mybir.ActivationFunctionType.Relu)
                        dst = Zsh[l][c][g0:g0 + 4 * P, :].rearrange(
                            "(w p) f -> p w f", p=P)
                        nc.sync.dma_start(dst, zrow[:])
                    else:
                        for wl in range(nwin):
                            rows = min(P, rows_tot - wl * P)
                            nc.scalar.activation(
                                zrow[:rows, wl, :], ps[:rows, wl, :],
                                mybir.ActivationFunctionType.Relu)
                            nc.sync.dma_start(
                                Zsh[l][c][g0 + wl * P:g0 + wl * P + rows, :],
                                zrow[:rows, wl, :])
                nc.gpsimd.collective_compute(
                    "AllGather", mybir.AluOpType.bypass,
                    replica_groups=[list(range(NC))],
                    ins=[Zsh[l][c].opt()], outs=[Z[l + 1][c].opt()])

            # ---- layer-1 dense (replicated; chunk-major output) ---------
            for c in range(NCHUNK):
                csz, cst = CSZ[c], CSTART[c]
                for r in range(NC):
                    col0 = r * NPC + cst
                    out0 = r * csz
                    for g0 in range(0, csz, 4 * P):
                        rows_tot = min(4 * P, csz - g0)
                        nwin = (rows_tot + P - 1) // P
                        xt_tile = xs.tile([P, 4 * P], FP16, tag="xt")
                        nc.sync.dma_start(xt_tile[:, :rows_tot],
                                          x_t[:, col0 + g0:col0 + g0 + rows_tot])
                        ps = psp.tile([P, 4, F], F32, tag="dense", bufs=2)
                        dense_group(
                            ps,
                            [lambda wl, rows: xt_tile[:, wl * P:wl * P + rows]],
                            Ws[0], b4s[0], nwin, rows_tot)
                        zrow = zrp.tile([P, 4, F], FP16, tag="zrow")
                        if rows_tot == 4 * P:
                            nc.scalar.activation(
                                zrow[:].rearrange("p w f -> p (w f)"),
                                ps[:].rearrange("p w f -> p (w f)"),
                                mybir.ActivationFunctionType.Relu)
                            dst = Z[0][c][out0 + g0:out0 + g0 + 4 * P, :]\
                                .rearrange("(w p) f -> p w f", p=P)
                            nc.sync.dma_start(dst, zrow[:])
                        else:
                            for wl in range(nwin):
                                rows = min(P, rows_tot - wl * P)
                                nc.scalar.activation(
                                    zrow[:rows, wl, :], ps[:rows, wl, :],
                                    mybir.ActivationFunctionType.Relu)
                                nc.sync.dma_start(
                                    Z[0][c][out0 + g0 + wl * P:
                                            out0 + g0 + wl * P + rows, :],
                                    zrow[:rows, wl, :])

            # ---- spmm layers --------------------------------------------
            for l in range(cfg.N_LAYERS):
                si = 0
                for cd, (w0, w1) in enumerate(WCH):
                    # gather runs for this chunk: one per src chunk
                    stages = []  # (tile, first_subtile, n_sub)
                    for q in range(NCHUNK):
                        run = int(nsub[q, w0:w1].sum())
                        src_ap = Z[l][q][:]
                        done = si
                        end = si + run
                        while done < end:
                            k = min(CALL // P, end - done)
                            st = stg.tile([P, CALL // P, F], FP16, tag="gst")
                            nc.gpsimd.dma_gather(
                                out_ap=st[:, :k, :], in_ap=src_ap,
                                idxs_ap=idx[:, done * 8:(done + k) * 8],
                                num_idxs=k * P, num_idxs_reg=k * P,
                                elem_size=F, queue_num=qrr[0] % 4)
                            qrr[0] += 1
                            stages.append((st, done, k))
                            done += k
                        si = end

                    # overlap previous chunk's dense+AllGather with this
                    # chunk's gathers (emitted after them on the Pool queue)
                    if l < cfg.N_LAYERS - 1 and cd >= 1:
                        emit_dense_chunk(l, cd - 1)

                    # consumption: the 4 src passes accumulate directly in
                    # PSUM (per-window has_written group opened on the first
                    # pass, closed on the last); ACT copies PSUM -> HT so the
                    # DVE queue never stalls on the matmul tail
                    nwinc = w1 - w0
                    groups = [(g, min(g + 4, nwinc))
                              for g in range(0, nwinc, 4)]
                    sg = 0
                    sii = stages[0][1]
                    for q in range(NCHUNK):
                        pst = [psp.tile([P, gw1 - gw0, P], F32, tag="spmm",
                                        bufs=5, name="ps_spmm")
                               for (gw0, gw1) in groups]
                        for w in range(w0, w1):
                            gi = (w - w0) // 4
                            wi = (w - w0) % 4
                            ns = int(nsub[q, w])
                            for k in range(ns):
                                st, s0, sk = stages[sg]
                                loc = sii - s0
                                if ABLATE == 'no_oh':
                                    oh = oh_dummy
                                else:
                                    oh = ohp.tile([P, P], FP16, tag="oh")
                                    # one-hot row e = w[e]*(iota==dstoff[e])
                                    nc.vector.tensor_scalar(
                                        oh[:], iota[:],
                                        dstoff[:, sii:sii + 1],
                                        wcol[:, sii:sii + 1],
                                        mybir.AluOpType.is_equal,
                                        mybir.AluOpType.mult)
                                if ABLATE != 'no_mm':
                                    # HT4[q][:, w] += G^T(e,f) @ OH(e,seg)
                                    nc.tensor.matmul(
                                        pst[gi][:, wi, :], st[:, loc, :],
                                        oh[:], start=(k == 0),
                                        stop=(k == ns - 1))
                                sii += 1
                                if sii - s0 >= sk:
                                    sg += 1
                        for (gw0, gw1), ps in zip(groups, pst):
                            if ABLATE == 'no_mm':
                                continue
                            c0 = CSTART[cd] + gw0 * P
                            cols = min((gw1 - gw0) * P, NPC - c0)
                            src = ps[:].rearrange(
                                "p w f -> p (w f)")[:, :cols]
                            nc.scalar.activation(
                                HT4[q][:, c0:c0 + cols], src,
                                mybir.ActivationFunctionType.Copy)

                if l < cfg.N_LAYERS - 1:
                    emit_dense_chunk(l, NCHUNK - 1)

            # ---- readout -----------------------------------------------
            if dump_ht:
                htf = res.tile([P, NPC], F32, name="htf")
                nc.vector.tensor_copy(htf[:], HT4[0][:])
                for q in range(1, NCHUNK):
                    nc.vector.tensor_tensor(htf[:], htf[:], HT4[q][:],
                                            mybir.AluOpType.add)
                nc.sync.dma_start(ht_out_d[:], htf[:])
                nrow0 = NC * CSZ[0]
                for r0 in range(0, nrow0, P):
                    rr = min(P, nrow0 - r0)
                    zt = zrp.tile([P, F], FP16, tag="z0dump", name="z0dump")
                    nc.sync.dma_start(zt[:rr, :], Z[0][0][r0:r0 + rr, :])
                    nc.sync.dma_start(z0_out_d[r0:r0 + rr, :], zt[:rr, :])
            gpart = res.tile([P, 1], F32)
            gparts = res.tile([P, NCHUNK], F32, name="gparts")
            for q in range(NCHUNK):
                nc.vector.tensor_reduce(gparts[:, q:q + 1], HT4[q][:],
                                        mybir.AxisListType.X,
                                        mybir.AluOpType.add)
            nc.vector.tensor_reduce(gpart[:], gparts[:], mybir.AxisListType.X,
                                    mybir.AluOpType.add)
            nc.sync.dma_start(g_in[:], gpart[:])
            nc.gpsimd.collective_compute(
                "AllReduce", mybir.AluOpType.add,
                replica_groups=[list(range(NC))],
                ins=[g_in.opt()], outs=[g_out.opt()])
            gsum = res.tile([P, 1], F32)
            nc.sync.dma_start(gsum[:], g_out[:])
            gf = res.tile([P, 1], FP16)
            nc.scalar.activation(gf[:], gsum[:],
                                 mybir.ActivationFunctionType.Copy,
                                 scale=1.0 / N)
            ps1 = psp.tile([32, 1], F32, tag="head", bufs=1)
            nc.tensor.matmul(ps1[:], Wf1[:], gf[:], start=True, stop=True)
            o_col = res.tile([32, 1], FP16)
            nc.scalar.activation(o_col[:], ps1[:],
                                 mybir.ActivationFunctionType.Relu,
                                 bias=bf1[:])
            ps2 = psp.tile([1, 1], F32, tag="head", bufs=1)
            nc.tensor.matmul(ps2[:], o_col[:], Wf2[:], start=True, stop=True)
            r_sb = res.tile([1, 1], F32)
            nc.scalar.activation(r_sb[:], ps2[:],
                                 mybir.ActivationFunctionType.Sigmoid,
                                 bias=bf2[:])
            nc.sync.dma_start(out_d[:], r_sb[:])

    nc.compile()
    return nc


def make_in_maps(cfg, inputs, sched, per_core):
    x = np.asarray(inputs["x"])
    x_t = np.ascontiguousarray(x.T).astype(np.float16)

    def b4(b):
        return np.tile(np.asarray(b).reshape(1, F), (1, 4)).astype(np.float16)

    common = dict(
        x_t=x_t,
        W1=np.asarray(inputs["W1"]).astype(np.float16),
        W2=np.asarray(inputs["W2"]).astype(np.float16),
        W3=np.asarray(inputs["W3"]).astype(np.float16),
        b4_1=b4(inputs["b1"]),
        b4_2=b4(inputs["b2"]),
        b4_3=b4(inputs["b3"]),
        Wf1=np.asarray(inputs["Wf1"]).astype(np.float16),
        bf1=np.asarray(inputs["bf1"]).reshape(32, 1).astype(np.float16),
        Wf2=np.asarray(inputs["Wf2"]).astype(np.float16),
        bf2=np.asarray(inputs["bf2"]).reshape(1, 1).astype(np.float16),
        iotat=np.tile(np.arange(P, dtype=np.float16), (P, 1)),
    )
    in_maps = []
    for c in range(NC):
        m = dict(common)
        m.update(per_core[c])
        in_maps.append(m)
    return in_maps


_CACHE = {}


def kernel(x, edge_src, edge_dst, edge_weight, W1, b1, W2, b2, W3, b3,
           Wf1, bf1, Wf2, bf2):
    inputs = dict(x=x, W1=W1, b1=b1, W2=W2, b2=b2, W3=W3, b3=b3,
                  Wf1=Wf1, bf1=bf1, Wf2=Wf2, bf2=bf2)
    cfg = Cfg(N=int(np.asarray(x).shape[0]))
    sched, per_core = preprocess(cfg, edge_src, edge_dst, edge_weight)
    key = (cfg.N, sched["S"],
           tuple(np.asarray(sched["nsub"]).ravel().tolist()))
    if key in _CACHE:
        nc = _CACHE[key]
    else:
        nc = build(cfg, sched)
        _CACHE[key] = nc
    in_maps = make_in_maps(cfg, inputs, sched, per_core)
    res = run_bass_kernel_spmd(nc, in_maps, core_ids=list(range(NC)))
    out = np.asarray(res.results[0]["out"], dtype=np.float32)
    return out.reshape(()).astype(np.float32)
